# revision 1
# baseline (speedup 1.0000x reference)
"""Trainium2 Bass kernel for dynamic low-pass filter decomposition.

Module: global-avg-pool -> 1x1 conv -> BN -> softmax over 3x3 taps gives a
per-(sample, group) 3x3 kernel; applied as a reflect-padded depthwise conv
over x; returns (low, x - low).

Sharding: data-parallel over batch n=8 across 8 NeuronCores (1 sample/core).

Per-core layout: partition p = h*64 + c (h = row-half of the image, c =
channel).  Each partition holds 98 rows x 192 cols of its (channel, half)
with one halo row above/below (reflection resolved at DMA time by source row
choice) plus a 1-element front/back pad so tap-shifted views stay in bounds.

The 9-tap weighted sum runs on the TensorEngine as 9 diagonal fp32r matmuls
per 512-column chunk accumulating in PSUM; ScalarE copies low out of PSUM,
VectorE computes high = x - low and fixes the reflect columns at w=0/191.
The softmax "kernel generation" runs on-device from exact per-ST partial
sums (fp32), with BN folded into the 1x1 conv weights on the host.
"""
import sys
import os

sys.path.insert(0, "/opt/trn_rl_repo")

import numpy as np
from contextlib import ExitStack

import concourse.bass as bass
import concourse.tile as tile
from concourse import bacc, mybir
from concourse.bass_utils import run_bass_kernel_spmd

dt = mybir.dt
f32 = dt.float32

KS = 3
GROUP = 8
IC = 64
BN_EPS = 1e-5
N = 8
H = W = 192
RH = 96                 # rows per half-image
NB = 98 * W             # buffer elems per partition (98 rows of 192)
PAD = 1                 # front pad elems (also 1 at the back)
NST = 6                 # input-phase tiles
STW = 3072              # cols per input tile
CH = 512                # cols per chunk (one PSUM bank)
ST_ROWS = [16, 16, 16, 16, 16, 8, 8]   # compute super-tile heights (rows)


def _build_program():
    """Trace the SPMD Bass program (same for every core)."""
    nc = bacc.Bacc("TRN2", target_bir_lowering=False, debug=False,
                   num_devices=N)

    x_d = nc.dram_tensor("x", [64, H, W], dt.float32r, kind="ExternalInput")
    at_d = nc.dram_tensor("at128", [128, 72], f32, kind="ExternalInput")
    b_d = nc.dram_tensor("b72", [72, 1], f32, kind="ExternalInput")
    r9_d = nc.dram_tensor("r9", [72, 9], f32, kind="ExternalInput")
    g_d = nc.dram_tensor("g728", [72, 8], f32, kind="ExternalInput")
    h_d = nc.dram_tensor("h8128", [8, 128], f32, kind="ExternalInput")
    eye_d = nc.dram_tensor("eye", [128, 128], f32, kind="ExternalInput")
    low_d = nc.dram_tensor("low", [64, H, W], f32, kind="ExternalOutput")
    high_d = nc.dram_tensor("high", [64, H, W], f32, kind="ExternalOutput")

    xt_dram = x_d.ap()

    def dram_flat(tensor, base, inner):
        """Flat (128, inner) AP over DRAM: partition p = c*2 + h covers
        x.flat[p*18432 + base : ... + inner].  Flat leading-dim-128 APs get
        the full 16-engine DMA spray (~305 GB/s); (h,c)-interleaved ones
        only engage 2 engines (~53 GB/s measured)."""
        return bass.AP(tensor, base, [[RH * W, 128], [1, inner]])

    with tile.TileContext(nc) as tc, ExitStack() as ctx:
        cpool = ctx.enter_context(tc.tile_pool(name="consts", bufs=1))
        xpool = ctx.enter_context(tc.tile_pool(name="x", bufs=1))
        wpool = ctx.enter_context(tc.tile_pool(name="w", bufs=1))
        spool = ctx.enter_context(tc.tile_pool(name="stage", bufs=3))

        # ---- x ST loads FIRST (queue FIFO position = landing time);
        # consts/halos after, so they don't delay the reduces ----
        xt = xpool.tile([128, PAD + NB + 1], dt.float32r)
        partials_v = wpool.tile([128, NST // 2], f32)
        partials_a = wpool.tile([128, NST // 2], f32)
        rscratch = wpool.tile([128, STW], f32)
        for s in range(NST):
            a = PAD + W + STW * s
            eng = nc.sync if s < 3 else nc.scalar
            eng.dma_start(xt[:, a:a + STW],
                          dram_flat(xt_dram.tensor, STW * s, STW))
        for s in range(NST):
            a = PAD + W + STW * s
            if s < 3:
                nc.vector.tensor_reduce(partials_v[:, s:s + 1],
                                        xt[:, a:a + STW].bitcast(f32),
                                        axis=mybir.AxisListType.X,
                                        op=mybir.AluOpType.add)
            else:
                nc.scalar.activation(rscratch[:],
                                     xt[:, a:a + STW].bitcast(f32),
                                     mybir.ActivationFunctionType.Copy,
                                     accum_out=partials_a[:, s - 3:s - 2])

        # ---- constant + halo loads (needed from the weight chain on) ----
        at_s = cpool.tile([128, 72], f32)
        b_s = cpool.tile([72, 1], f32)
        r9_s = cpool.tile([72, 9], f32)
        g_s = cpool.tile([72, 8], f32)
        h_s = cpool.tile([8, 128], f32)
        eye_s = cpool.tile([128, 128], f32)
        for t, d in ((at_s, at_d), (b_s, b_d), (r9_s, r9_d), (g_s, g_d),
                     (h_s, h_d), (eye_s, eye_d)):
            nc.scalar.dma_start(t[:], d.ap())
        # halo row 0 <- image rows {1 (reflect), 95}[h]
        nc.sync.dma_start(xt[:, PAD:PAD + W],
                          bass.AP(xt_dram.tensor, W,
                                  [[H * W, 64], [94 * W, 2], [1, W]]))
        # halo row 97 <- image rows {96, 190 (reflect)}[h]
        nc.sync.dma_start(xt[:, PAD + 97 * W:PAD + 98 * W],
                          bass.AP(xt_dram.tensor, 96 * W,
                                  [[H * W, 64], [94 * W, 2], [1, W]]))

        # ---- weight generation ----
        sum_v = wpool.tile([128, 1], f32)
        nc.vector.tensor_reduce(sum_v[:], partials_v[:],
                                axis=mybir.AxisListType.X,
                                op=mybir.AluOpType.add)
        sum_a = wpool.tile([128, 1], f32)
        nc.vector.tensor_reduce(sum_a[:], partials_a[:],
                                axis=mybir.AxisListType.X,
                                op=mybir.AluOpType.add)
        sum128 = wpool.tile([128, 1], f32)
        nc.vector.tensor_add(sum128[:], sum_v[:], sum_a[:])
        with tc.tile_pool(name="wpsum", bufs=1,
                          space=bass.MemorySpace.PSUM) as wpsum:
            lf_p = wpsum.tile([72, 1], f32, tag="lf")
            nc.tensor.matmul(lf_p[:], at_s[:], sum128[:])
            e72 = wpool.tile([72, 1], f32)
            nc.scalar.activation(e72[:], lf_p[:],
                                 mybir.ActivationFunctionType.Exp,
                                 bias=b_s[:, 0:1], scale=1.0)
            rhsw = wpool.tile([72, 9], f32)
            nc.vector.tensor_scalar_mul(rhsw[:], r9_s[:], e72[:, 0:1])
            w89_p = wpsum.tile([8, 9], f32, tag="w89")
            nc.tensor.matmul(w89_p[:], g_s[:], rhsw[:])
            s8 = wpool.tile([8, 1], f32)
            nc.vector.tensor_reduce(s8[:], w89_p[:],
                                    axis=mybir.AxisListType.X,
                                    op=mybir.AluOpType.add)
            r8 = wpool.tile([8, 1], f32)
            nc.vector.reciprocal(r8[:], s8[:])
            w89s = wpool.tile([8, 9], f32)
            nc.vector.tensor_scalar_mul(w89s[:], w89_p[:], r8[:, 0:1])
            wbig_p = wpsum.tile([128, 9], f32, tag="wbig")
            nc.tensor.matmul(wbig_p[:], h_s[:], w89s[:])
            w128 = wpool.tile([128, 9], f32)
            nc.scalar.copy(w128[:], wbig_p[:])

        # diagonal weight matrices, one tile per tap (separate tiles so
        # the first matmul only waits for its own diagonal); scalars read
        # straight from PSUM so PE needn't wait for the w128 SBUF copy
        diag = [wpool.tile([128, 128], dt.float32r, name=f"diag{k}")
                for k in range(9)]
        for k in range(9):
            nc.vector.tensor_scalar_mul(diag[k][:], eye_s[:],
                                        wbig_p[:, k:k + 1])

        # ---- main loop ----
        with tc.tile_pool(name="psum", bufs=8,
                          space=bass.MemorySpace.PSUM) as psum:
            r0 = 0
            for s, rows in enumerate(ST_ROWS):
                stw = rows * W
                nch = stw // CH
                base = PAD + W + r0 * W
                acc = [psum.tile([128, CH], f32, tag="acc", name=f"acc{s}_{i}")
                       for i in range(nch)]
                taps = range(9) if s % 2 == 0 else range(8, -1, -1)
                taps = list(taps)
                for k in taps:
                    di, dj = k // 3, k % 3
                    shift = (di - 1) * W + (dj - 1)
                    for ch in range(nch):
                        off = base + CH * ch + shift
                        nc.tensor.matmul(acc[ch][:], diag[k][:],
                                         xt[:, off:off + CH],
                                         start=(k == taps[0]),
                                         stop=(k == taps[-1]))
                low_st = spool.tile([128, stw], f32, tag="low",
                                    padded_shape=[128, 3072])
                for ch in range(nch):
                    dst = low_st[:, CH * ch:CH * (ch + 1)]
                    if ch % 2 == 0:
                        nc.scalar.copy(dst, acc[ch][:])
                    else:
                        nc.vector.tensor_copy(dst, acc[ch][:])
                # edge-column fixes (reflect at w=0 and w=191), both columns
                # per op via a stride-(wr-wl) length-2 inner dim
                out_ap = low_st[:, 0:stw].rearrange(
                    "p (r w) -> p r w", w=W)[:, :, 0:W:W - 1]
                for k in range(9):
                    di, dj = k // 3, k % 3
                    wl = (1, 0, 1)[dj]
                    wr = (190, 191, 190)[dj]
                    vb = PAD + (r0 + di) * W + wl
                    view = xt[:, vb:vb + rows * W].bitcast(f32).rearrange(
                        "p (r w) -> p r w", w=W)[:, :, 0:wr - wl + 1:wr - wl]
                    if k == 0:
                        nc.vector.tensor_scalar_mul(out_ap, view,
                                                    w128[:, 0:1])
                    else:
                        nc.vector.scalar_tensor_tensor(
                            out_ap, view, w128[:, k:k + 1], out_ap,
                            op0=mybir.AluOpType.mult,
                            op1=mybir.AluOpType.add)
                high_st = spool.tile([128, stw], f32, tag="high",
                                     padded_shape=[128, 3072])
                nc.vector.tensor_tensor(high_st[:],
                                        xt[:, base:base + stw].bitcast(f32),
                                        low_st[:],
                                        op=mybir.AluOpType.subtract)
                nc.scalar.dma_start(
                    dram_flat(low_d.ap().tensor, r0 * W, stw), low_st[:])
                nc.sync.dma_start(
                    dram_flat(high_d.ap().tensor, r0 * W, stw), high_st[:])
                r0 += rows

    nc.compile()
    return nc


def _enable_ldw_opt():
    """walrus emits one LDWEIGHTS per matmul with --enable-ldw-opt=false
    (72us of PE time for our 330 matmuls, mostly redundant reloads of the
    same diagonal).  Rewrite the flag on the compiler command line."""
    import concourse.bass_utils as BU
    if getattr(BU, "_ldw_patched", False):
        return
    orig = BU.run_command

    def patched(cmd, *a, **kw):
        cmd = [c.replace("--enable-ldw-opt=false", "--enable-ldw-opt=true")
               if isinstance(c, str) else c for c in cmd]
        return orig(cmd, *a, **kw)

    BU.run_command = patched
    BU._ldw_patched = True
    # bir_verify_and_optimise captured run_command at def time? (no - it
    # resolves the module global at call time, so the wrap is enough)


_nc_cache = None


def _get_program():
    global _nc_cache
    if _nc_cache is None:
        _enable_ldw_opt()
        _nc_cache = _build_program()
    return _nc_cache


def _host_consts(conv_w, bn_gamma, bn_beta, bn_mean, bn_var):
    s_a = bn_gamma / np.sqrt(bn_var + BN_EPS)
    b72 = (bn_beta - bn_mean * s_a).astype(np.float32).reshape(72, 1)
    A = (conv_w * s_a[:, None]) / np.float32(H * W)
    p = np.arange(128)
    at128 = np.ascontiguousarray(A.T[p // 2]).astype(np.float32)  # (128, 72)
    oc = np.arange(72)
    r9 = (oc[:, None] % 9 == np.arange(9)[None, :]).astype(np.float32)
    g728 = (oc[:, None] // 9 == np.arange(8)[None, :]).astype(np.float32)
    h8128 = (np.arange(8)[:, None] == (p[None, :] // 16)).astype(np.float32)
    eye = np.eye(128, dtype=np.float32)
    return dict(at128=at128, b72=b72, r9=r9, g728=g728, h8128=h8128, eye=eye)


def kernel(x, conv_w, bn_gamma, bn_beta, bn_mean, bn_var):
    x = np.ascontiguousarray(np.asarray(x, dtype=np.float32))
    consts = _host_consts(np.asarray(conv_w, np.float32),
                          np.asarray(bn_gamma, np.float32),
                          np.asarray(bn_beta, np.float32),
                          np.asarray(bn_mean, np.float32),
                          np.asarray(bn_var, np.float32))
    nc = _get_program()
    in_maps = [dict(x=x[i], **consts) for i in range(N)]
    res = run_bass_kernel_spmd(nc, in_maps, list(range(N))).results
    low = np.stack([res[i]["low"] for i in range(N)])
    high = np.stack([res[i]["high"] for i in range(N)])
    return low, high


if __name__ == "__main__":
    rng = np.random.default_rng(0)
    demo = dict(
        x=rng.standard_normal((N, IC, H, W), dtype=np.float32),
        conv_w=rng.standard_normal((72, 64)).astype(np.float32),
        bn_gamma=np.ones(72, np.float32),
        bn_beta=np.zeros(72, np.float32),
        bn_mean=rng.standard_normal(72).astype(np.float32) * 0.1,
        bn_var=rng.uniform(0.5, 1.5, 72).astype(np.float32),
    )
    low, high = kernel(**demo)
    print("ok", low.shape, high.shape)



# revision 8
# speedup vs baseline: 1.0374x; 1.0374x over previous
"""Trainium2 Bass kernel for dynamic low-pass filter decomposition.

Module: global-avg-pool -> 1x1 conv -> BN -> softmax over 3x3 taps gives a
per-(sample, group) 3x3 kernel; applied as a reflect-padded depthwise conv
over x; returns (low, x - low).

Sharding: data-parallel over batch n=8 across 8 NeuronCores (1 sample/core).

v2 (fp16): all HBM traffic in fp16 (load 4.7 MB, store 2x4.5 MB per core);
x lives in SBUF as 98 rows (96 image rows + 2 reflected halo rows) with a
196-element row stride whose pad columns hold the w-reflect values, so every
3x3 tap is a plain shifted view and NO edge fixups are needed.  The 9-tap
weighted sum is split across engines: TensorE does the 6 odd-shift taps as
diagonal fp16 matmuls into PSUM (2-row / 384-col chunks), VectorE adds the 3
even-shift taps (fp16 2x mode) after ScalarE copies PSUM->SBUF, then VectorE
computes high = x - low.  Softmax weight generation runs on-device from
per-load-tile partial sums; warmup matmuls keep the PE HAM clock at 2.4 GHz
through the load phase.
"""
import sys

sys.path.insert(0, "/opt/trn_rl_repo")

import numpy as np
from contextlib import ExitStack

import concourse.bass as bass
import concourse.tile as tile
from concourse import bacc, mybir
from concourse.bass_utils import run_bass_kernel_spmd

dt = mybir.dt
f32 = dt.float32
f16 = dt.float16

KS = 3
GROUP = 8
IC = 64
BN_EPS = 1e-5
N = 8
H = W = 192
RH = 96                  # rows per half-image
RS = 196                 # SBUF row stride (2 pad cols + 192 + 2 pad cols)
NROWS = 98               # 96 image rows + top/bottom halo
XLEN = NROWS * RS + 4    # xt elems per partition (+4 so shifted views slice)
CTR = RS + 2             # offset of image row 0, col 0 inside xt
OLEN = RH * W            # dense output elems per partition (18432)

# load tiles (rows each, descending so the last partial-sum lands early)
LD_ROWS = [16, 16, 16, 16, 12, 10, 6, 4]
# compute groups (rows each; last small to shrink the serial tail)
ST_ROWS = [16, 16, 16, 16, 16, 12, 4]
CROWS = 2                # rows per PSUM chunk (384 fp32 cols of a bank)

PE_TAPS = [0, 2, 3, 5, 6, 8]   # odd elem shift -> no DVE 2x, keep on PE
DVE_TAPS = [1, 4, 7]           # even elem shift -> DVE 2x candidates
WARM_PER_ST = 12               # HAM warmup matmuls issued per load tile


def _shift(k):
    di, dj = k // 3, k % 3
    return (di - 1) * RS + (dj - 1)


def _build_program():
    """Trace the SPMD Bass program (same for every core)."""
    nc = bacc.Bacc("TRN2", target_bir_lowering=False, debug=False,
                   num_devices=N)

    x_d = nc.dram_tensor("x", [64, H, W], f16, kind="ExternalInput")
    at_d = nc.dram_tensor("at128", [128, 72], f32, kind="ExternalInput")
    b_d = nc.dram_tensor("b72", [72, 1], f32, kind="ExternalInput")
    r9_d = nc.dram_tensor("r9", [72, 9], f32, kind="ExternalInput")
    g_d = nc.dram_tensor("g728", [72, 8], f32, kind="ExternalInput")
    h_d = nc.dram_tensor("h8128", [8, 128], f32, kind="ExternalInput")
    eye_d = nc.dram_tensor("eye", [128, 128], f16, kind="ExternalInput")
    low_d = nc.dram_tensor("low", [64, H, W], f16, kind="ExternalOutput")
    high_d = nc.dram_tensor("high", [64, H, W], f16, kind="ExternalOutput")

    xt_dram = x_d.ap()

    def dram_flat(tensor, base, inner):
        """Flat (128, inner) AP over DRAM: partition p = c*2 + h covers
        x.flat[p*18432 + base : ... + inner].  Flat leading-dim-128 APs get
        the full 16-engine DMA spray; interleaved ones only engage 2."""
        return bass.AP(tensor, base, [[RH * W, 128], [1, inner]])

    with tile.TileContext(nc) as tc, ExitStack() as ctx:
        cpool = ctx.enter_context(tc.tile_pool(name="consts", bufs=1))
        xpool = ctx.enter_context(tc.tile_pool(name="x", bufs=1))
        wpool = ctx.enter_context(tc.tile_pool(name="w", bufs=1))
        opool = ctx.enter_context(tc.tile_pool(name="out", bufs=1))

        xt = xpool.tile([128, XLEN], f16)
        low = opool.tile([128, OLEN], f16)
        high = opool.tile([128, OLEN], f16)

        # ---- consts on sync queue FIRST (tiny; eye feeds PE warmup),
        # then x tiles interleaved sync/scalar so they land in order ----
        at_s = cpool.tile([128, 72], f32)
        b_s = cpool.tile([72, 1], f32)
        r9_s = cpool.tile([72, 9], f32)
        g_s = cpool.tile([72, 8], f32)
        h_s = cpool.tile([8, 128], f32)
        eye_s = cpool.tile([128, 128], f16)
        for t, d in ((eye_s, eye_d), (at_s, at_d), (b_s, b_d), (r9_s, r9_d),
                     (g_s, g_d), (h_s, h_d)):
            nc.sync.dma_start(t[:], d.ap())

        def row_view(tile_, row0, nrows, extra=0):
            """[[RS, nrows], [1, W]] view of xt starting at image col 0 of
            buffer row row0, shifted by `extra` elements (may cross rows)."""
            s0 = row0 * RS + 2 + extra
            return tile_[:, s0:s0 + nrows * RS].rearrange(
                "p (r w) -> p r w", w=RS)[:, :, 0:W]

        nld = len(LD_ROWS)
        ld_r0 = [int(v) for v in np.cumsum([0] + LD_ROWS)[:-1]]
        for s, (r0, rows) in enumerate(zip(ld_r0, LD_ROWS)):
            eng = nc.sync if s % 2 == 0 else nc.scalar
            eng.dma_start(row_view(xt, r0 + 1, rows),
                          dram_flat(xt_dram.tensor, r0 * W, rows * W))
        # halo row 0 <- image rows {1 (reflect), 95}[h]
        nc.sync.dma_start(
            xt[:, 2:2 + W],
            bass.AP(xt_dram.tensor, W, [[H * W, 64], [94 * W, 2], [1, W]]))
        # halo row 97 <- image rows {96, 190 (reflect)}[h]
        nc.scalar.dma_start(
            xt[:, 97 * RS + 2:97 * RS + 2 + W],
            bass.AP(xt_dram.tensor, 96 * W,
                    [[H * W, 64], [94 * W, 2], [1, W]]))

        # ---- partial sums for the global mean, one per load tile ----
        rowsum = wpool.tile([128, 96], f32)
        partials_a = wpool.tile([128, nld // 2], f32)
        rscratch = wpool.tile([128, 3072], f16)
        for s, (r0, rows) in enumerate(zip(ld_r0, LD_ROWS)):
            src = row_view(xt, r0 + 1, rows)
            if s % 2 == 0:
                nc.vector.tensor_reduce(rowsum[:, r0:r0 + rows], src,
                                        axis=mybir.AxisListType.X,
                                        op=mybir.AluOpType.add)
            else:
                rsv = rscratch[:, :rows * W].rearrange(
                    "p (r w) -> p r w", w=W)
                nc.scalar.activation(rsv, src,
                                     mybir.ActivationFunctionType.Copy,
                                     accum_out=partials_a[:, s // 2:s // 2 + 1])

        # ---- HAM warmup: keep PE busy through the load phase ----
        with tc.tile_pool(name="wpsum", bufs=1,
                          space=bass.MemorySpace.PSUM) as wpsum:
            warm = wpsum.tile([128, 384], f32, tag="warm",
                              padded_shape=[128, 512])
            for s, (r0, rows) in enumerate(zip(ld_r0, LD_ROWS)):
                for i in range(WARM_PER_ST):
                    nc.tensor.matmul(warm[:], eye_s[:],
                                     row_view(xt, r0 + 1, CROWS), start=True,
                                     stop=True)

            # ---- w-reflect pad columns (after all loads + halos) ----
            xcols = xt[:, 0:NROWS * RS].rearrange("p (r w) -> p r w", w=RS)
            nc.vector.tensor_copy(xcols[:, :, 1:2], xcols[:, :, 3:4])
            nc.vector.tensor_copy(xcols[:, :, 194:195], xcols[:, :, 192:193])

            # ---- weight generation (all fp32, as the math demands) ----
            sum_v = wpool.tile([128, 1], f32)
            nc.vector.tensor_reduce(sum_v[:], rowsum[:],
                                    axis=mybir.AxisListType.X,
                                    op=mybir.AluOpType.add)
            sum_a = wpool.tile([128, 1], f32)
            nc.vector.tensor_reduce(sum_a[:], partials_a[:],
                                    axis=mybir.AxisListType.X,
                                    op=mybir.AluOpType.add)
            sum128 = wpool.tile([128, 1], f32)
            nc.vector.tensor_add(sum128[:], sum_v[:], sum_a[:])

            lf_p = wpsum.tile([72, 1], f32, tag="lf")
            nc.tensor.matmul(lf_p[:], at_s[:], sum128[:])
            e72 = wpool.tile([72, 1], f32)
            nc.scalar.activation(e72[:], lf_p[:],
                                 mybir.ActivationFunctionType.Exp,
                                 bias=b_s[:, 0:1], scale=1.0)
            rhsw = wpool.tile([72, 9], f32)
            nc.vector.tensor_scalar_mul(rhsw[:], r9_s[:], e72[:, 0:1])
            w89_p = wpsum.tile([8, 9], f32, tag="w89")
            nc.tensor.matmul(w89_p[:], g_s[:], rhsw[:])
            s8 = wpool.tile([8, 1], f32)
            nc.vector.tensor_reduce(s8[:], w89_p[:],
                                    axis=mybir.AxisListType.X,
                                    op=mybir.AluOpType.add)
            r8 = wpool.tile([8, 1], f32)
            nc.vector.reciprocal(r8[:], s8[:])
            w89s = wpool.tile([8, 9], f32)
            nc.vector.tensor_scalar_mul(w89s[:], w89_p[:], r8[:, 0:1])
            wbig_p = wpsum.tile([128, 9], f32, tag="wbig")
            nc.tensor.matmul(wbig_p[:], h_s[:], w89s[:])
            w128 = wpool.tile([128, 9], f32)
            nc.scalar.copy(w128[:], wbig_p[:])

            # diagonal fp16 weight matrices for the PE taps (scalars read
            # straight from PSUM so PE needn't wait for the w128 copy)
            diag = {}
            for k in PE_TAPS:
                diag[k] = wpool.tile([128, 128], f16, name=f"diag{k}")
            for k in PE_TAPS:
                nc.vector.tensor_scalar_mul(diag[k][:], eye_s[:],
                                            wbig_p[:, k:k + 1])

        # ---- main loop ----
        with tc.tile_pool(name="psum", bufs=8,
                          space=bass.MemorySpace.PSUM) as psum:
            r0 = 0
            for s, rows in enumerate(ST_ROWS):
                nch = rows // CROWS
                acc = [psum.tile([128, CROWS * W], f32, tag="acc",
                                 name=f"acc{s}_{i}", padded_shape=[128, 512])
                       for i in range(nch)]
                taps = PE_TAPS if s % 2 == 0 else PE_TAPS[::-1]
                for k in taps:
                    for ch in range(nch):
                        nc.tensor.matmul(acc[ch][:], diag[k][:],
                                         row_view(xt, r0 + 1 + ch * CROWS,
                                                  CROWS, extra=_shift(k)),
                                         start=(k == taps[0]),
                                         stop=(k == taps[-1]))
                for ch in range(nch):
                    o0 = (r0 + ch * CROWS) * W
                    nc.scalar.copy(low[:, o0:o0 + CROWS * W], acc[ch][:])
                ost = r0 * W
                lview = low[:, ost:ost + rows * W].rearrange(
                    "p (r w) -> p r w", w=W)
                for k in DVE_TAPS:
                    nc.vector.scalar_tensor_tensor(
                        lview, row_view(xt, r0 + 1, rows, extra=_shift(k)),
                        w128[:, k:k + 1], lview,
                        op0=mybir.AluOpType.mult,
                        op1=mybir.AluOpType.add)
                hview = high[:, ost:ost + rows * W].rearrange(
                    "p (r w) -> p r w", w=W)
                nc.vector.tensor_tensor(hview, row_view(xt, r0 + 1, rows),
                                        lview,
                                        op=mybir.AluOpType.subtract)
                nc.scalar.dma_start(
                    dram_flat(low_d.ap().tensor, ost, rows * W),
                    low[:, ost:ost + rows * W])
                nc.sync.dma_start(
                    dram_flat(high_d.ap().tensor, ost, rows * W),
                    high[:, ost:ost + rows * W])
                r0 += rows

    nc.compile()
    return nc


def _enable_ldw_opt():
    """walrus emits one LDWEIGHTS per matmul with --enable-ldw-opt=false
    (redundant reloads of the same diagonal).  Rewrite the flag on the
    compiler command line."""
    import concourse.bass_utils as BU
    if getattr(BU, "_ldw_patched", False):
        return
    orig = BU.run_command

    def patched(cmd, *a, **kw):
        cmd = [c.replace("--enable-ldw-opt=false", "--enable-ldw-opt=true")
               if isinstance(c, str) else c for c in cmd]
        return orig(cmd, *a, **kw)

    BU.run_command = patched
    BU._ldw_patched = True


_nc_cache = None


def _get_program():
    global _nc_cache
    if _nc_cache is None:
        # NOTE: ldw-opt stays OFF — walrus rejects fp16 (FWL-path) LDWEIGHTS
        # with --enable-ldw-opt=true; fp16 FWL loads are cheap (~32 cyc) and
        # overlap matmuls via the background weight buffer.
        _nc_cache = _build_program()
    return _nc_cache


def _host_consts(conv_w, bn_gamma, bn_beta, bn_mean, bn_var):
    s_a = bn_gamma / np.sqrt(bn_var + BN_EPS)
    b72 = (bn_beta - bn_mean * s_a).astype(np.float32).reshape(72, 1)
    A = (conv_w * s_a[:, None]) / np.float32(H * W)
    p = np.arange(128)
    at128 = np.ascontiguousarray(A.T[p // 2]).astype(np.float32)  # (128, 72)
    oc = np.arange(72)
    r9 = (oc[:, None] % 9 == np.arange(9)[None, :]).astype(np.float32)
    g728 = (oc[:, None] // 9 == np.arange(8)[None, :]).astype(np.float32)
    h8128 = (np.arange(8)[:, None] == (p[None, :] // 16)).astype(np.float32)
    eye = np.eye(128, dtype=np.float16)
    return dict(at128=at128, b72=b72, r9=r9, g728=g728, h8128=h8128, eye=eye)


def _prepare(x, conv_w, bn_gamma, bn_beta, bn_mean, bn_var):
    x16 = np.ascontiguousarray(np.asarray(x, dtype=np.float16))
    consts = _host_consts(np.asarray(conv_w, np.float32),
                          np.asarray(bn_gamma, np.float32),
                          np.asarray(bn_beta, np.float32),
                          np.asarray(bn_mean, np.float32),
                          np.asarray(bn_var, np.float32))
    return [dict(x=x16[i], **consts) for i in range(N)]


def _collect(res):
    low = np.stack([res[i]["low"] for i in range(N)]).astype(np.float32)
    high = np.stack([res[i]["high"] for i in range(N)]).astype(np.float32)
    return low, high


def kernel(x, conv_w, bn_gamma, bn_beta, bn_mean, bn_var):
    in_maps = _prepare(x, conv_w, bn_gamma, bn_beta, bn_mean, bn_var)
    nc = _get_program()
    res = run_bass_kernel_spmd(nc, in_maps, list(range(N))).results
    return _collect(res)


if __name__ == "__main__":
    rng = np.random.default_rng(0)
    demo = dict(
        x=rng.standard_normal((N, IC, H, W), dtype=np.float32),
        conv_w=rng.standard_normal((72, 64)).astype(np.float32),
        bn_gamma=np.ones(72, np.float32),
        bn_beta=np.zeros(72, np.float32),
        bn_mean=rng.standard_normal(72).astype(np.float32) * 0.1,
        bn_var=rng.uniform(0.5, 1.5, 72).astype(np.float32),
    )
    low, high = kernel(**demo)
    print("ok", low.shape, high.shape)


# revision 24
# speedup vs baseline: 1.1915x; 1.1486x over previous
"""Trainium2 Bass kernel for dynamic low-pass filter decomposition.

Module: global-avg-pool -> 1x1 conv -> BN -> softmax over 3x3 taps gives a
per-(sample, group) 3x3 kernel; applied as a reflect-padded depthwise conv
over x; returns (low, x - low).

Sharding: data-parallel over batch n=8 across 8 NeuronCores (1 sample/core).

All HBM traffic is fp16.  x is pre-padded on the host into [128, 98*196]
per core (partition p = c*2 + h; 96 image rows + 2 reflected halo rows per
half; each row stored [., padL, 192 cols, padR, .] with the w-reflect values
in the pads) so loads are pure flat DMA and every 3x3 tap on device is a
plain shifted flat view with NO edge fixups.  low/high keep the same padded
row stride in SBUF/DRAM (pad lanes compute garbage, host strips them), so
the whole main loop is flat 512-col chunks:

  PE     7 taps as diagonal fp16 matmuls into PSUM (per-mm LDWEIGHTS is
         fully hidden behind the previous matmul: measured 164ns cadence)
  ScalarE PSUM->SBUF copy (fp32 -> fp16)
  VectorE 2 even-shift taps as tensor_scalar_mul (4x) + tensor_tensor add
         (2x)  [scalar_tensor_tensor only has a 1x uop: measured], then
         high = x - low (2x)

Weight generation runs on-device from per-load-tile partial sums; warmup
matmuls keep the PE HAM clock at 2.4 GHz through the load phase.
"""
import sys

sys.path.insert(0, "/opt/trn_rl_repo")

import numpy as np
from contextlib import ExitStack

import concourse.bass as bass
import concourse.tile as tile
from concourse import bacc, mybir
from concourse.bass_utils import run_bass_kernel_spmd

dt = mybir.dt
f32 = dt.float32
f16 = dt.float16

KS = 3
GROUP = 8
IC = 64
BN_EPS = 1e-5
N = 8
H = W = 192
RH = 96                  # rows per half-image
RS = 196                 # row stride (2 pad cols + 192 + 2 pad cols)
NROWS = 98               # 96 image rows + top/bottom halo
XPLEN = NROWS * RS       # padded x elems per partition (19208)
XB = 4 + RS              # xt idx of out elem 0 (skip front slack + halo row)
OLEN = RH * RS           # padded out elems per partition (18816)
CH = 512                 # cols per PSUM chunk
NCHUNK = (OLEN + CH - 1) // CH          # 37 (last chunk 384)
GROUPS = [6, 6, 6, 6, 6, 6, 1]          # chunks per compute group

# load tiles (rows each, descending so the last partial-sum lands early)
LD_ROWS = [16, 16, 16, 16, 12, 10, 6, 4]

PE_TAPS = [0, 2, 3, 4, 5, 6, 8]   # diagonal fp16 matmuls
DVE_TAPS = [1, 7]                 # even shift: ts_mul(4x) + tt_add(2x)
WARM_PER_ST = 6                   # HAM warmup matmuls issued per load tile


def _shift(k):
    di, dj = k // 3, k % 3
    return (di - 1) * RS + (dj - 1)


def _build_program():
    """Trace the SPMD Bass program (same for every core)."""
    nc = bacc.Bacc("TRN2", target_bir_lowering=False, debug=False,
                   num_devices=N)

    x_d = nc.dram_tensor("x", [128, XPLEN], f16, kind="ExternalInput")
    at_d = nc.dram_tensor("at128", [128, 72], f32, kind="ExternalInput")
    b_d = nc.dram_tensor("b72", [72, 1], f32, kind="ExternalInput")
    r9_d = nc.dram_tensor("r9", [72, 9], f32, kind="ExternalInput")
    g_d = nc.dram_tensor("g728", [72, 8], f32, kind="ExternalInput")
    h_d = nc.dram_tensor("h8128", [8, 128], f32, kind="ExternalInput")
    eye_d = nc.dram_tensor("eye", [128, 128], f16, kind="ExternalInput")
    low_d = nc.dram_tensor("low", [128, OLEN], f16, kind="ExternalOutput")
    high_d = nc.dram_tensor("high", [128, OLEN], f16, kind="ExternalOutput")

    def dram_flat(tensor, base, inner, pitch):
        """Flat (128, inner) DRAM AP: full 16-engine DMA spray."""
        return bass.AP(tensor, base, [[pitch, 128], [1, inner]])

    with tile.TileContext(nc) as tc, ExitStack() as ctx:
        cpool = ctx.enter_context(tc.tile_pool(name="consts", bufs=1))
        xpool = ctx.enter_context(tc.tile_pool(name="x", bufs=1))
        wpool = ctx.enter_context(tc.tile_pool(name="w", bufs=1))
        opool = ctx.enter_context(tc.tile_pool(name="out", bufs=1))
        tpool = ctx.enter_context(tc.tile_pool(name="tmp", bufs=2))

        xt = xpool.tile([128, 4 + XPLEN + 4], f16)
        low = opool.tile([128, OLEN], f16)
        high = opool.tile([128, OLEN], f16)

        # ---- consts first on sync queue (tiny; eye feeds PE warmup) ----
        at_s = cpool.tile([128, 72], f32)
        b_s = cpool.tile([72, 1], f32)
        r9_s = cpool.tile([72, 9], f32)
        g_s = cpool.tile([72, 8], f32)
        h_s = cpool.tile([8, 128], f32)
        eye_s = cpool.tile([128, 128], f16)
        for t, d in ((eye_s, eye_d), (at_s, at_d), (b_s, b_d), (r9_s, r9_d),
                     (g_s, g_d), (h_s, h_d)):
            nc.sync.dma_start(t[:], d.ap())

        # ---- x loads: flat, spread over 4 DMA queues ----
        ld_q = [nc.sync, nc.scalar, nc.gpsimd]
        nld = len(LD_ROWS)
        ld_r0 = [int(v) for v in np.cumsum([0] + LD_ROWS)[:-1]]
        for s, (r0, rows) in enumerate(zip(ld_r0, LD_ROWS)):
            a = (r0 + 1) * RS
            ld_q[s % 3].dma_start(
                xt[:, 4 + a:4 + a + rows * RS],
                dram_flat(x_d.ap().tensor, a, rows * RS, XPLEN))
        # halo rows (host-prepped reflections)
        nc.sync.dma_start(xt[:, 4:4 + RS],
                          dram_flat(x_d.ap().tensor, 0, RS, XPLEN))
        nc.scalar.dma_start(xt[:, 4 + 97 * RS:4 + 98 * RS],
                            dram_flat(x_d.ap().tensor, 97 * RS, RS, XPLEN))

        def row_view(row0, nrows, extra=0):
            """[[RS, nrows], [1, W]] image-cols view from buffer row row0."""
            s0 = 4 + row0 * RS + 2 + extra
            return xt[:, s0:s0 + nrows * RS].rearrange(
                "p (r w) -> p r w", w=RS)[:, :, 0:W]

        # ---- partial sums for the global mean, one per load tile ----
        ndv = sum(r for s, r in enumerate(LD_ROWS) if s % 2 == 0)
        rowsum = wpool.tile([128, ndv], f32)
        partials_a = wpool.tile([128, nld // 2], f32)
        rscratch = wpool.tile([128, 3072], f16)
        dv0 = 0
        for s, (r0, rows) in enumerate(zip(ld_r0, LD_ROWS)):
            src = row_view(r0 + 1, rows)
            if s % 2 == 0:
                nc.vector.tensor_reduce(rowsum[:, dv0:dv0 + rows], src,
                                        axis=mybir.AxisListType.X,
                                        op=mybir.AluOpType.add)
                dv0 += rows
            else:
                rsv = rscratch[:, :rows * W].rearrange(
                    "p (r w) -> p r w", w=W)
                nc.scalar.activation(rsv, src,
                                     mybir.ActivationFunctionType.Copy,
                                     accum_out=partials_a[:, s // 2:s // 2 + 1])

        # ---- HAM warmup: keep PE busy through the load phase ----
        with tc.tile_pool(name="wpsum", bufs=1,
                          space=bass.MemorySpace.PSUM) as wpsum:
            warm = wpsum.tile([128, 512], f32, tag="warm")
            for s, (r0, rows) in enumerate(zip(ld_r0, LD_ROWS)):
                a = 4 + (r0 + 1) * RS
                for i in range(WARM_PER_ST):
                    nc.tensor.matmul(warm[:], eye_s[:], xt[:, a:a + 512],
                                     start=True, stop=True)

            # ---- weight generation (all fp32, as the math demands) ----
            sum_v = wpool.tile([128, 1], f32)
            nc.vector.tensor_reduce(sum_v[:], rowsum[:],
                                    axis=mybir.AxisListType.X,
                                    op=mybir.AluOpType.add)
            sum_a = wpool.tile([128, 1], f32)
            nc.vector.tensor_reduce(sum_a[:], partials_a[:],
                                    axis=mybir.AxisListType.X,
                                    op=mybir.AluOpType.add)
            sum128 = wpool.tile([128, 1], f32)
            nc.vector.tensor_add(sum128[:], sum_v[:], sum_a[:])

            lf_p = wpsum.tile([72, 1], f32, tag="lf")
            nc.tensor.matmul(lf_p[:], at_s[:], sum128[:])
            e72 = wpool.tile([72, 1], f32)
            nc.scalar.activation(e72[:], lf_p[:],
                                 mybir.ActivationFunctionType.Exp,
                                 bias=b_s[:, 0:1], scale=1.0)
            rhsw = wpool.tile([72, 9], f32)
            nc.vector.tensor_scalar_mul(rhsw[:], r9_s[:], e72[:, 0:1])
            w89_p = wpsum.tile([8, 9], f32, tag="w89")
            nc.tensor.matmul(w89_p[:], g_s[:], rhsw[:])
            s8 = wpool.tile([8, 1], f32)
            nc.vector.tensor_reduce(s8[:], w89_p[:],
                                    axis=mybir.AxisListType.X,
                                    op=mybir.AluOpType.add)
            r8 = wpool.tile([8, 1], f32)
            nc.vector.reciprocal(r8[:], s8[:])
            w89s = wpool.tile([8, 9], f32)
            nc.vector.tensor_scalar_mul(w89s[:], w89_p[:], r8[:, 0:1])
            wbig_p = wpsum.tile([128, 9], f32, tag="wbig")
            nc.tensor.matmul(wbig_p[:], h_s[:], w89s[:])
            w128 = wpool.tile([128, 9], f32)
            nc.scalar.copy(w128[:], wbig_p[:])

            # diagonal fp16 weight matrices for the PE taps (scalars read
            # straight from PSUM so PE needn't wait for the w128 copy)
            diag = {}
            for k in PE_TAPS:
                diag[k] = wpool.tile([128, 128], f16, name=f"diag{k}")
            for k in PE_TAPS:
                nc.vector.tensor_scalar_mul(diag[k][:], eye_s[:],
                                            wbig_p[:, k:k + 1])

        # ---- main loop: flat 512-col chunks over the padded out layout ----
        with tc.tile_pool(name="psum", bufs=8,
                          space=bass.MemorySpace.PSUM) as psum:
            c0 = 0
            for s, nch in enumerate(GROUPS):
                g0 = c0 * CH
                glen = min(OLEN, (c0 + nch) * CH) - g0
                acc = []
                for i in range(nch):
                    cl = min(CH, OLEN - (c0 + i) * CH)
                    acc.append(psum.tile([128, cl], f32, tag="acc",
                                         name=f"acc{s}_{i}",
                                         padded_shape=[128, 512]))
                taps = PE_TAPS if s % 2 == 0 else PE_TAPS[::-1]
                for k in taps:
                    for i in range(nch):
                        a = XB + (c0 + i) * CH + _shift(k)
                        nc.tensor.matmul(acc[i][:], diag[k][:],
                                         xt[:, a:a + acc[i].shape[1]],
                                         start=(k == taps[0]),
                                         stop=(k == taps[-1]))
                for i in range(nch):
                    o = (c0 + i) * CH
                    nc.scalar.copy(low[:, o:o + acc[i].shape[1]], acc[i][:])
                for k in DVE_TAPS:
                    # scalar_tensor_tensor only has a 1x uop; ts_mul (4x) +
                    # tt add (2x) is faster for fp16 despite two passes
                    tmp = tpool.tile([128, 3072], f16, tag="tmp")
                    a = XB + g0 + _shift(k)
                    nc.vector.tensor_scalar_mul(tmp[:, :glen],
                                                xt[:, a:a + glen],
                                                w128[:, k:k + 1])
                    nc.vector.tensor_tensor(low[:, g0:g0 + glen],
                                            low[:, g0:g0 + glen],
                                            tmp[:, :glen],
                                            op=mybir.AluOpType.add)
                nc.vector.tensor_tensor(high[:, g0:g0 + glen],
                                        xt[:, XB + g0:XB + g0 + glen],
                                        low[:, g0:g0 + glen],
                                        op=mybir.AluOpType.subtract)
                nc.scalar.dma_start(
                    dram_flat(low_d.ap().tensor, g0, glen, OLEN),
                    low[:, g0:g0 + glen])
                nc.sync.dma_start(
                    dram_flat(high_d.ap().tensor, g0, glen, OLEN),
                    high[:, g0:g0 + glen])
                c0 += nch

    nc.compile()
    return nc


_nc_cache = None


def _get_program():
    global _nc_cache
    if _nc_cache is None:
        # NOTE: ldw-opt stays OFF (walrus rejects 16-bit LDWEIGHTS with
        # --enable-ldw-opt=true); per-mm LDWEIGHTS is fully hidden behind
        # the previous matmul (measured 164ns cadence for 384-col mms).
        _nc_cache = _build_program()
    return _nc_cache


def _host_consts(conv_w, bn_gamma, bn_beta, bn_mean, bn_var):
    s_a = bn_gamma / np.sqrt(bn_var + BN_EPS)
    b72 = (bn_beta - bn_mean * s_a).astype(np.float32).reshape(72, 1)
    A = (conv_w * s_a[:, None]) / np.float32(H * W)
    p = np.arange(128)
    at128 = np.ascontiguousarray(A.T[p // 2]).astype(np.float32)  # (128, 72)
    oc = np.arange(72)
    r9 = (oc[:, None] % 9 == np.arange(9)[None, :]).astype(np.float32)
    g728 = (oc[:, None] // 9 == np.arange(8)[None, :]).astype(np.float32)
    h8128 = (np.arange(8)[:, None] == (p[None, :] // 16)).astype(np.float32)
    eye = np.eye(128, dtype=np.float16)
    return dict(at128=at128, b72=b72, r9=r9, g728=g728, h8128=h8128, eye=eye)


def _pad_x(x16):
    """(n, 64, 192, 192) fp16 -> (n, 128, 98*196): 96 rows split into two
    halves stacked in the partition dim, one reflected halo row above and
    below each half, and each row stored as [., padL, 192 cols, padR, .]
    so 3x3 taps on device are plain shifted flat views."""
    n = x16.shape[0]
    xp = np.zeros((n, 64, 2, NROWS, RS), dtype=np.float16)
    xp[:, :, :, 1:97, 2:194] = x16.reshape(n, 64, 2, RH, W)
    xp[:, :, 0, 0, 2:194] = x16[:, :, 1]        # reflect of row -1
    xp[:, :, 1, 0, 2:194] = x16[:, :, 95]       # halo above bottom half
    xp[:, :, 0, 97, 2:194] = x16[:, :, 96]      # halo below top half
    xp[:, :, 1, 97, 2:194] = x16[:, :, 190]     # reflect of row 192
    xp[..., 1] = xp[..., 3]                     # reflect of col -1
    xp[..., 194] = xp[..., 192]                 # reflect of col 192
    return np.ascontiguousarray(xp.reshape(n, 128, XPLEN))


def _prepare(x, conv_w, bn_gamma, bn_beta, bn_mean, bn_var):
    x16 = np.asarray(x, dtype=np.float16)
    xp = _pad_x(x16)
    consts = _host_consts(np.asarray(conv_w, np.float32),
                          np.asarray(bn_gamma, np.float32),
                          np.asarray(bn_beta, np.float32),
                          np.asarray(bn_mean, np.float32),
                          np.asarray(bn_var, np.float32))
    return [dict(x=xp[i], **consts) for i in range(N)]


def _unpad(a):
    """[128, 96*196] padded-rows -> (64, 192, 192) fp32."""
    return a.reshape(64, 2, RH, RS)[..., 2:194].reshape(
        64, H, W).astype(np.float32)


def _collect(res):
    low = np.stack([_unpad(res[i]["low"]) for i in range(N)])
    high = np.stack([_unpad(res[i]["high"]) for i in range(N)])
    return low, high


def kernel(x, conv_w, bn_gamma, bn_beta, bn_mean, bn_var):
    in_maps = _prepare(x, conv_w, bn_gamma, bn_beta, bn_mean, bn_var)
    nc = _get_program()
    res = run_bass_kernel_spmd(nc, in_maps, list(range(N))).results
    return _collect(res)


if __name__ == "__main__":
    rng = np.random.default_rng(0)
    demo = dict(
        x=rng.standard_normal((N, IC, H, W), dtype=np.float32),
        conv_w=rng.standard_normal((72, 64)).astype(np.float32),
        bn_gamma=np.ones(72, np.float32),
        bn_beta=np.zeros(72, np.float32),
        bn_mean=rng.standard_normal(72).astype(np.float32) * 0.1,
        bn_var=rng.uniform(0.5, 1.5, 72).astype(np.float32),
    )
    low, high = kernel(**demo)
    print("ok", low.shape, high.shape)
